# revision 32
# baseline (speedup 1.0000x reference)
"""GAT-style GNN message passing on 8 Trainium2 NeuronCores (Bass/Tile).

Strategy (destination sharding, no collectives):
  Nodes are split into 8 contiguous destination shards (6250 each). Each core
  owns the edges whose *destination* lands in its shard, so the segment
  softmax and the weighted aggregation are core-local (no cross-device
  reduction at all).

  Per core, on device:
    Phase 0  full projection: xs = relu(x) @ W for ALL nodes (redundantly on
             every core; x is tiny vs. the edge work) with the per-node
             attention logits a_src/a_dst folded into the same matmul as
             extra columns.  Rows [xs | a_src | a_dst] are written to a bf16
             DRAM table with 768-byte rows for dma_gather.
    Phase 1  per destination tile of 128 nodes (49 tiles): the tile's edges
             (sorted by dest, padded to a fixed chunk capacity) are processed
             as 20 chunks of 128 edges:
               - dma_gather fetches [xs|a_src] of each edge's source node
                 (two calls per tile: source index is int16, so the table is
                 addressed in two halves),
               - per-edge logits alpha = a_src[src] + a_dst[dst] + a_edge are
                 assembled with two accumulating matmuls per chunk
                 (edge_attr @ w_edge_att, and one-hot(dst) @ a_dst_tile) plus
                 a DVE add of the gathered a_src,
               - p = exp(leaky_relu(alpha)) (softmax max-subtraction is
                 skipped: logits are O(1) and softmax is shift invariant),
               - messages p * xs[src] and the softmax denominators are
                 aggregated with one accumulating matmul per chunk against
                 the one-hot destination selection matrix,
               - out = aggregated / (segsum + eps) + bias.

  The chunk structure is input-independent (fixed per-tile capacities with
  5-sigma headroom + index padding), so the compiled NEFF never depends on
  the actual edge values.  If a pathological graph overflows a capacity, we
  fall back to a host computation (probability ~1e-5).
"""
import os
import numpy as np
import ml_dtypes

BF16 = ml_dtypes.bfloat16

# ---------------------------------------------------------------- problem dims
N, E = 50000, 800000
F_IN, EDGE_DIM, H, CH = 128, 64, 8, 32
HC = H * CH                      # 256
M_OUT = HC + 2 * H               # 272 proj cols: xs | a_src | a_dst
NEG_SLOPE = 0.2
N_CORES = 8


class Cfg:
    def __init__(self, n, e, n_cores, npc, ca, cb, split, nproj):
        self.n, self.e, self.n_cores, self.npc = n, e, n_cores, npc
        self.T = (npc + 127) // 128          # dest tiles per core
        self.npc_pad = self.T * 128
        self.CA, self.CB = ca, cb            # per-tile chunk capacity (A/B half)
        self.CT = ca + cb
        self.n_chunks = self.T * self.CT
        self.slots = self.n_chunks * 128
        self.split = split                   # first table row of the B half
        self.nproj = nproj                   # gather-table rows (mult of 2048)
        assert nproj % 2048 == 0 and nproj >= n
        assert nproj - split < 32768 and split <= 32768
        self.ROW = 384                       # bf16 elems per table row (768 B)
        self.gcols = self.T * self.CT * 8    # gather-idx cols ((CA+CB)*128/16)


CFG_FULL = Cfg(N, E, N_CORES, N // N_CORES, ca=12, cb=7, split=32768, nproj=51200)

_CACHE = {}


# =====================================================================
# Bass program
# =====================================================================
def _build(cfg):
    from concourse import bacc, tile, mybir
    from concourse.bass import AP

    f32, bf16 = mybir.dt.float32, mybir.dt.bfloat16
    f8 = mybir.dt.float8e4
    i16, u8 = mybir.dt.int16, mybir.dt.uint8

    nc = bacc.Bacc("TRN2", target_bir_lowering=False, debug=False,
                   num_devices=cfg.n_cores)

    # -------- I/O ----------------------------------------------------
    xT = nc.dram_tensor("xT", [128, cfg.nproj], bf16, kind="ExternalInput")
    wcat = nc.dram_tensor("wcat", [128, M_OUT], bf16, kind="ExternalInput")
    wedge = nc.dram_tensor("wedge", [64, H], bf16, kind="ExternalInput")
    bias_rep = nc.dram_tensor("bias_rep", [128, HC], f32, kind="ExternalInput")
    iota_row = nc.dram_tensor("iota_row", [128, 128], bf16,
                              kind="ExternalInput")
    iota_col = nc.dram_tensor("iota_col", [128, 1], u8, kind="ExternalInput")
    eps_c = nc.dram_tensor("eps_c", [128, 1], f32, kind="ExternalInput")
    gidx = nc.dram_tensor("gidx", [128, cfg.gcols], i16, kind="ExternalInput")
    dlane = nc.dram_tensor("dlane", [128, cfg.n_chunks], bf16,
                           kind="ExternalInput")
    dfree = nc.dram_tensor("dfree", [128, cfg.slots], u8, kind="ExternalInput")
    eaT = nc.dram_tensor("eaT", [64, cfg.slots], bf16, kind="ExternalInput")

    table = nc.dram_tensor("table", [cfg.nproj, cfg.ROW], bf16, kind="Internal")
    adst_d = nc.dram_tensor("adst_d", [cfg.npc_pad, H], bf16, kind="Internal")
    out_d = nc.dram_tensor("out", [cfg.npc_pad, HC], f32, kind="ExternalOutput")

    table_ap = table.ap()
    adst_ap = adst_d.ap()

    GRP = 16                          # proj chunks per table-write DMA
    n_pchunks = cfg.nproj // 128
    n_pgrp = n_pchunks // GRP

    with tile.TileContext(nc) as tc:
        # constants resident all along
        with tc.tile_pool(name="const", bufs=1) as cst:
            wcat_sb = cst.tile([128, M_OUT], bf16)
            wedge_sb = cst.tile([64, H], bf16)
            bias_sb = cst.tile([128, HC], f32)
            irow_sb = cst.tile([128, 128], bf16)
            icol_sb = cst.tile([128, 1], u8)
            eps_sb = cst.tile([128, 1], f32)
            gidx_sb = cst.tile([128, cfg.gcols], i16)
            dlane_sb = cst.tile([128, cfg.n_chunks], bf16)
            nc.sync.dma_start(out=wcat_sb[:, :], in_=wcat.ap()[:, :])
            nc.sync.dma_start(out=wedge_sb[:, :], in_=wedge.ap()[:, :])
            nc.sync.dma_start(out=bias_sb[:, :], in_=bias_rep.ap()[:, :])
            nc.sync.dma_start(out=irow_sb[:, :], in_=iota_row.ap()[:, :])
            nc.sync.dma_start(out=icol_sb[:, :], in_=iota_col.ap()[:, :])
            nc.sync.dma_start(out=eps_sb[:, :], in_=eps_c.ap()[:, :])
            nc.sync.dma_start(out=gidx_sb[:, :], in_=gidx.ap()[:, :])
            nc.sync.dma_start(out=dlane_sb[:, :], in_=dlane.ap()[:, :])

            # ---------------- phase 0: projection + table build ----------
            with (
                tc.tile_pool(name="p0x", bufs=3) as p0x,
                tc.tile_pool(name="p0r", bufs=3) as p0r,
                tc.tile_pool(name="p0ps", bufs=2, space="PSUM") as p0ps,
                tc.tile_pool(name="p0st", bufs=2) as p0st,
                tc.tile_pool(name="p0ad", bufs=2) as p0ad,
            ):
                for g in range(n_pgrp):
                    xt = p0x.tile([128, GRP * 128], bf16, tag="xt")
                    nc.sync.dma_start(
                        out=xt[:, :],
                        in_=xT.ap()[:, g * GRP * 128:(g + 1) * GRP * 128])
                    xr = p0r.tile([128, GRP * 128], bf16, tag="xr")
                    nc.scalar.activation(xr[:, :], xt[:, :],
                                         mybir.ActivationFunctionType.Relu)
                    stage = p0st.tile([128, GRP, cfg.ROW], bf16, tag="stage")
                    nc.gpsimd.memset(stage[:, :, M_OUT:], 0.0)
                    for k4 in range(0, GRP, 4):
                        ps4 = p0ps.tile([128, 4, 512], f32, tag="pproj",
                                        space="PSUM")
                        for k in range(4):
                            nc.tensor.matmul(ps4[:, k, 0:M_OUT],
                                             xr[:, (k4 + k) * 128:
                                                (k4 + k + 1) * 128],
                                             wcat_sb[:, :],
                                             start=True, stop=True)
                        if k4 % 16 == 12:
                            nc.scalar.copy(out=stage[:, k4:k4 + 4, 0:M_OUT],
                                           in_=ps4[:, :, 0:M_OUT])
                        else:
                            nc.vector.tensor_copy(
                                out=stage[:, k4:k4 + 4, 0:M_OUT],
                                in_=ps4[:, :, 0:M_OUT])
                    # table rows [g*GRP*128, (g+1)*GRP*128) ; row = 128*grp + p
                    nc.sync.dma_start(
                        out=AP(table_ap.tensor, g * GRP * 128 * cfg.ROW,
                               [[cfg.ROW, 128], [128 * cfg.ROW, GRP],
                                [1, cfg.ROW]]),
                        in_=stage[:, :, :])

            # ---------------- phase 1: per-dest-tile edge processing ------
            LA, LB = cfg.CA * 128, cfg.CB * 128
            CT, ROW = cfg.CT, cfg.ROW
            with (
                tc.tile_pool(name="gbuf", bufs=2) as pgb,
                tc.tile_pool(name="ea", bufs=2) as pea,
                tc.tile_pool(name="dfr", bufs=2) as pdf,
                tc.tile_pool(name="smat", bufs=2) as psm,
                tc.tile_pool(name="stmat", bufs=2) as pst,
                tc.tile_pool(name="msgp", bufs=2) as pmg,
                tc.tile_pool(name="adt", bufs=2) as pad,
                tc.tile_pool(name="alph", bufs=2) as pal,
                tc.tile_pool(name="osb", bufs=2) as pos,
                tc.tile_pool(name="ps1", bufs=2, space="PSUM") as ps1,
                tc.tile_pool(name="ps2", bufs=2, space="PSUM") as ps2,
            ):
                QCH = 6  # chunks per dma_gather call (<=768 idxs: SWDGE ring)
                for t in range(cfg.T):
                    gb = pgb.tile([128, CT, ROW], bf16, tag="gb")
                    c0 = t * (CT * 8)
                    for q in range(0, cfg.CA, QCH):
                        nch = min(QCH, cfg.CA - q)
                        nc.gpsimd.dma_gather(
                            gb[:, q:q + nch, :], table_ap[:, :],
                            gidx_sb[:, c0 + q * 8:c0 + (q + nch) * 8],
                            nch * 128, nch * 128, ROW)
                    cb0 = c0 + cfg.CA * 8
                    for q in range(0, cfg.CB, QCH):
                        nch = min(QCH, cfg.CB - q)
                        nc.gpsimd.dma_gather(
                            gb[:, cfg.CA + q:cfg.CA + q + nch, :],
                            table_ap[cfg.split:, :],
                            gidx_sb[:, cb0 + q * 8:cb0 + (q + nch) * 8],
                            nch * 128, nch * 128, ROW)

                    ea = pea.tile([64, CT * 128], bf16, tag="ea")
                    nc.sync.dma_start(
                        out=ea[:, :],
                        in_=eaT.ap()[:, t * CT * 128:(t + 1) * CT * 128])
                    df = pdf.tile([128, CT * 128], u8, tag="df")
                    nc.sync.dma_start(
                        out=df[:, :],
                        in_=dfree.ap()[:, t * CT * 128:(t + 1) * CT * 128])
                    adt = pad.tile([128, H], bf16, tag="adt")
                    nc.sync.dma_start(
                        out=adt[:, :],
                        in_=table_ap[t * 128:(t + 1) * 128, HC + H:M_OUT])

                    # one-hot selection matrices (bf16 0/1)
                    S = psm.tile([128, CT, 128], bf16, tag="S")
                    irow_b = irow_sb[:, :]
                    nc.vector.tensor_tensor(
                        out=S[:, :, :],
                        in0=dlane_sb[:, t * CT:(t + 1) * CT]
                            .to_broadcast([128, CT, 128]),
                        in1=AP(irow_b.tensor, irow_b.offset,
                               [irow_b.ap[0], [0, CT], irow_b.ap[1]]),
                        op=mybir.AluOpType.is_equal)
                    ST = pst.tile([128, CT * 128], bf16, tag="ST")
                    nc.vector.tensor_tensor(
                        out=ST[:, :],
                        in0=icol_sb[:, 0:1].to_broadcast([128, CT * 128]),
                        in1=df[:, :], op=mybir.AluOpType.is_equal)

                    # per-edge logit pieces: a_edge + a_dst via matmuls
                    psa = ps1.tile([128, CT * H], f32, tag="psa", space="PSUM")
                    for c in range(CT):
                        nc.tensor.matmul(psa[:, c * H:(c + 1) * H],
                                         ea[:, c * 128:(c + 1) * 128],
                                         wedge_sb[:, :], start=True, stop=False)
                        nc.tensor.matmul(psa[:, c * H:(c + 1) * H],
                                         ST[:, c * 128:(c + 1) * 128],
                                         adt[:, :], start=False, stop=True)

                    alpha = pal.tile([128, CT, H], f32, tag="alpha")
                    nc.vector.tensor_tensor(out=alpha[:, :, :],
                                            in0=AP(psa[:, :].tensor,
                                                   psa[:, :].offset,
                                                   [psa[:, :].ap[0], [H, CT],
                                                    [1, H]]),
                                            in1=gb[:, :, HC:HC + H],
                                            op=mybir.AluOpType.add)
                    alpha_b = alpha[:, :, :]
                    alpha_flat = AP(alpha_b.tensor, alpha_b.offset,
                                    [alpha_b.ap[0], [1, CT * H]])
                    # leaky_relu(x) = max(0.2*x, x); then p = exp(...),
                    # written channel-expanded by ACT so the message multiply
                    # is fully packed bf16 (DVE 2x mode).
                    lrel = pal.tile([128, CT * H], f32, tag="lrel")
                    nc.vector.scalar_tensor_tensor(
                        out=lrel[:, :], in0=alpha_flat, scalar=NEG_SLOPE,
                        in1=alpha_flat, op0=mybir.AluOpType.mult,
                        op1=mybir.AluOpType.max)
                    pexp = pal.tile([128, CT, H, CH], bf16, tag="pexp")
                    lr_b = lrel[:, :]
                    nc.scalar.activation(
                        pexp[:, :, :, :],
                        AP(lr_b.tensor, lr_b.offset,
                           [lr_b.ap[0], [H, CT], [1, H], [0, CH]]),
                        mybir.ActivationFunctionType.Exp)

                    # msgp rows: [ p*xs (256) | p (8) ] per chunk
                    MB = HC + H  # 264
                    mg = pmg.tile([128, CT * MB], bf16, tag="mg")
                    mg_b = mg[:, :]
                    gb_b = gb[:, :, :]
                    pe_b = pexp[:, :, :, :]
                    nc.vector.tensor_tensor(
                        out=AP(mg_b.tensor, mg_b.offset,
                               [mg_b.ap[0], [MB, CT], [CH, H], [1, CH]]),
                        in0=AP(gb_b.tensor, gb_b.offset,
                               [gb_b.ap[0], [ROW, CT], [CH, H], [1, CH]]),
                        in1=pe_b,
                        op=mybir.AluOpType.mult)
                    nc.vector.tensor_copy(
                        out=AP(mg_b.tensor, mg_b.offset + HC,
                               [mg_b.ap[0], [MB, CT], [1, H]]),
                        in_=AP(pe_b.tensor, pe_b.offset,
                               [pe_b.ap[0], [H * CH, CT], [CH, H]]))

                    pso = ps2.tile([128, MB], f32, tag="pso", space="PSUM")
                    for c in range(CT):
                        nc.tensor.matmul(pso[:, :], S[:, c, :],
                                         mg[:, c * MB:(c + 1) * MB],
                                         start=(c == 0), stop=(c == CT - 1))

                    seg = pal.tile([128, H], f32, tag="seg")
                    nc.vector.tensor_scalar_add(seg[:, :], pso[:, HC:MB],
                                                1e-16)
                    rec = pal.tile([128, H], f32, tag="rec")
                    nc.vector.reciprocal(out=rec[:, :], in_=seg[:, :])
                    ot = pos.tile([128, HC], f32, tag="ot")
                    ot_b = ot[:, :]
                    pso_b = pso[:, :]
                    nc.vector.tensor_tensor(
                        out=AP(ot_b.tensor, ot_b.offset,
                               [ot_b.ap[0], [CH, H], [1, CH]]),
                        in0=AP(pso_b.tensor, pso_b.offset,
                               [pso_b.ap[0], [CH, H], [1, CH]]),
                        in1=rec[:, :].to_broadcast([128, H, CH]),
                        op=mybir.AluOpType.mult)
                    nc.vector.tensor_tensor(out=ot[:, :], in0=ot[:, :],
                                            in1=bias_sb[:, :],
                                            op=mybir.AluOpType.add)
                    nc.sync.dma_start(out=out_d.ap()[t * 128:(t + 1) * 128, :],
                                      in_=ot[:, :])

    nc.compile()
    return nc


# =====================================================================
# Host-side data prep
# =====================================================================
def _wrap16(vals):
    """dma_gather index layout: element i lives at [i % 16, i // 16],
    replicated across the 8 groups of 16 partitions."""
    L = vals.shape[0]
    w = vals.reshape(L // 16, 16).T.astype(np.int16)       # [16, L/16]
    return np.tile(w, (8, 1))                              # [128, L/16]


def _prep_core(cfg, c, x_bf, src, dst, edge_attr_bf, base, cnt):
    """Build the per-core input dict. Returns None on capacity overflow."""
    npc = cfg.npc
    lo = c * npc
    hi = min(cfg.n, lo + npc)
    eid = np.nonzero((dst >= lo) & (dst < hi))[0]
    d_local = (dst[eid] - lo).astype(np.int64)
    tile_id = d_local >> 7
    dst_off = (d_local & 127).astype(np.uint8)

    # permutation: own nodes first, then the rest; gather idx = inv[src]
    own = np.arange(lo, hi, dtype=np.int64)
    rest = np.concatenate([np.arange(0, lo, dtype=np.int64),
                           np.arange(hi, cfg.n, dtype=np.int64)])
    perm = np.concatenate([own, np.zeros(cfg.npc_pad - own.size, np.int64),
                           rest,
                           np.zeros(cfg.nproj - cfg.npc_pad - rest.size,
                                    np.int64)])
    inv = np.empty(cfg.n, np.int64)
    inv[own] = np.arange(own.size)
    inv[rest] = cfg.npc_pad + np.arange(rest.size)

    r = inv[src[eid]]
    half = (r >= cfg.split).astype(np.int64)
    order = np.lexsort((half, tile_id))
    eid, r, half, tile_id, dst_off = (eid[order], r[order], half[order],
                                      tile_id[order], dst_off[order])

    # per (tile, half) slot placement
    key = tile_id * 2 + half
    counts = np.bincount(key, minlength=cfg.T * 2)
    capA, capB = cfg.CA * 128, cfg.CB * 128
    caps = np.tile([capA, capB], cfg.T)
    if np.any(counts > caps):
        return None
    block_base = np.zeros(cfg.T * 2, np.int64)
    block_base[0::2] = np.arange(cfg.T) * cfg.CT * 128
    block_base[1::2] = block_base[0::2] + capA
    first = np.zeros(cfg.T * 2, np.int64)
    first[1:] = np.cumsum(counts)[:-1]
    slots = block_base[key] + (np.arange(eid.size) - first[key])

    gvals = np.zeros(cfg.slots, np.int64)
    gvals[slots] = np.where(half == 0, r, r - cfg.split)
    doff = np.full(cfg.slots, 255, np.uint8)
    doff[slots] = dst_off
    ea = np.zeros((64, cfg.slots), BF16)
    ea[:, slots] = edge_attr_bf[eid].T

    # gather idx tensor: per tile, A block then B block
    g = np.zeros((128, cfg.gcols), np.int16)
    gv = gvals.reshape(cfg.n_chunks, 128)
    for t in range(cfg.T):
        s0 = t * cfg.CT * 128
        cb = t * cfg.CT * 8
        g[:, cb:cb + capA // 16] = _wrap16(gvals[s0:s0 + capA])
        g[:, cb + capA // 16:cb + (capA + capB) // 16] = \
            _wrap16(gvals[s0 + capA:s0 + capA + capB])

    return {
        "xT": np.ascontiguousarray(x_bf[:, perm]),
        "gidx": np.ascontiguousarray(g),
        "dlane": np.ascontiguousarray(
            doff.reshape(cfg.n_chunks, 128).T.astype(BF16)),
        "dfree": np.ascontiguousarray(np.broadcast_to(doff, (128, cfg.slots))),
        "eaT": np.ascontiguousarray(ea),
    }


def _host_reference(x, edge_index, edge_attr, W, att_src, att_dst, W_edge,
                    att_edge, bias):
    """numpy fallback (only used if a tile capacity overflows)."""
    src, dst = edge_index[0].astype(np.int64), edge_index[1].astype(np.int64)
    n = x.shape[0]
    xr = np.maximum(x, 0.0)
    xs = (xr @ W).reshape(n, H, CH)
    a_src = np.einsum("nhc,hc->nh", xs, att_src)
    a_dst = np.einsum("nhc,hc->nh", xs, att_dst)
    e_proj = (edge_attr @ W_edge).reshape(-1, H, CH)
    a_edge = np.einsum("ehc,hc->eh", e_proj, att_edge)
    alpha = a_src[src] + a_dst[dst] + a_edge
    alpha = np.where(alpha >= 0, alpha, NEG_SLOPE * alpha)
    seg_max = np.full((n, H), -np.inf, np.float32)
    np.maximum.at(seg_max, dst, alpha)
    seg_max = np.where(np.isfinite(seg_max), seg_max, 0.0)
    p = np.exp(alpha - seg_max[dst])
    seg_sum = np.zeros((n, H), np.float32)
    np.add.at(seg_sum, dst, p)
    p = p / (seg_sum[dst] + 1e-16)
    out = np.zeros((n, H, CH), np.float32)
    np.add.at(out, dst, p[:, :, None] * xs[src].reshape(-1, H, CH))
    return (out.reshape(n, HC) + bias).astype(np.float32)


LAST_EXEC_NS = None


def prepare_in_maps(cfg, inputs):
    x = np.asarray(inputs["x"], np.float32)
    edge_index = np.asarray(inputs["edge_index"])
    edge_attr = np.asarray(inputs["edge_attr"], np.float32)
    W = np.asarray(inputs["W"], np.float32)
    att_src = np.asarray(inputs["att_src"], np.float32)
    att_dst = np.asarray(inputs["att_dst"], np.float32)
    W_edge = np.asarray(inputs["W_edge"], np.float32)
    att_edge = np.asarray(inputs["att_edge"], np.float32)
    bias = np.asarray(inputs["bias"], np.float32)

    # folded weights (replicated)
    W3 = W.reshape(F_IN, H, CH)
    wcat = np.concatenate(
        [W, np.einsum("fhc,hc->fh", W3, att_src),
         np.einsum("fhc,hc->fh", W3, att_dst)], axis=1).astype(BF16)
    wedge = np.einsum("dhc,hc->dh", W_edge.reshape(EDGE_DIM, H, CH),
                      att_edge).astype(BF16)
    bias_rep = np.ascontiguousarray(
        np.broadcast_to(bias.astype(np.float32), (128, HC)))
    iota_row = np.ascontiguousarray(np.broadcast_to(
        np.arange(128, dtype=np.float32).astype(BF16), (128, 128)))
    iota_col = np.arange(128, dtype=np.uint8).reshape(128, 1).copy()

    xT_bf = np.zeros((F_IN, cfg.nproj), BF16)
    xT_bf[:, :cfg.n] = x.T.astype(BF16)
    ea_bf = edge_attr.astype(BF16)
    src = edge_index[0].astype(np.int64)
    dst = edge_index[1].astype(np.int64)

    shared = {"wcat": np.ascontiguousarray(wcat), "wedge": wedge,
              "bias_rep": bias_rep, "iota_row": iota_row,
              "iota_col": iota_col,
              "eps_c": np.full((128, 1), 1e-16, np.float32)}
    in_maps = []
    for c in range(cfg.n_cores):
        m = _prep_core(cfg, c, xT_bf, src, dst, ea_bf, None, None)
        if m is None:
            return None  # capacity overflow -> host fallback
        m.update(shared)
        in_maps.append(m)
    return in_maps


def _run(cfg, inputs):
    global LAST_EXEC_NS
    from concourse.bass_utils import run_bass_kernel_spmd

    in_maps = prepare_in_maps(cfg, inputs)
    if in_maps is None:
        return None

    key = id(cfg)
    if key not in _CACHE:
        _CACHE[key] = _build(cfg)
    nc = _CACHE[key]

    res = run_bass_kernel_spmd(nc, in_maps, list(range(cfg.n_cores)))
    LAST_EXEC_NS = res.exec_time_ns

    out = np.empty((cfg.n, HC), np.float32)
    for c in range(cfg.n_cores):
        lo = c * cfg.npc
        hi = min(cfg.n, lo + cfg.npc)
        out[lo:hi] = res.results[c]["out"][:hi - lo]
    return out


def kernel(x, edge_index, edge_attr, W, att_src, att_dst, W_edge, att_edge,
           bias):
    inputs = dict(x=x, edge_index=edge_index, edge_attr=edge_attr, W=W,
                  att_src=att_src, att_dst=att_dst, W_edge=W_edge,
                  att_edge=att_edge, bias=bias)
    out = _run(CFG_FULL, inputs)
    if out is None:
        out = _host_reference(**{k: np.asarray(v, np.float32)
                                 if k != "edge_index" else np.asarray(v)
                                 for k, v in inputs.items()})
    return out
